# revision 14
# baseline (speedup 1.0000x reference)
"""Trainium2 Bass kernel for nn_CrossAttention (channel attention / XCA-style).

Sharding: 8 cores = 2 batches x 4 row-strips of 64 rows. Each core computes
its strip end-to-end; the attention score matrix S = k q^T and the q/k row
norms are partial-summed per core and AllReduced across the 4 cores of the
same batch (replica groups [[0..3],[4..7]]), after which every core finishes
softmax + attn@v + projection for its own strip.

Layouts inside a core (channels on partitions, 192 = 128+64 chunks):
  conv1x1 (bf16 matmul)  ->  pre [ch, (rows+2)*258] padded windows
  dwconv3x3 = 9 diagonal-stationary matmuls with free-dim-shifted operands
  q,k -> PE-transposed per 128-pos chunk -> gram matmuls accumulate S' = k q^T
  norms via squared-transpose x ones matmuls
  softmax on S'' (masked to per-head 24x24 blocks), attn@v + proj in fp32r.
"""

import sys
import numpy as np

if "/opt/trn_rl_repo" not in sys.path:
    sys.path.insert(0, "/opt/trn_rl_repo")

import ml_dtypes

BF = ml_dtypes.bfloat16

B = 2
C = 192
HEADS = 8
CH = C // HEADS  # 24
H = W = 256
PITCH = W + 2  # 258
NCORES = 8
RSTRIP = 64          # rows per core
RG = 16              # rows per group
NG = RSTRIP // RG    # 4
WINR = RG + 2        # 18
WIN = WINR * PITCH   # 4644
WINA = WIN + 2       # pre-window alloc (dwconv operand slop)
DWN = RG * PITCH     # 4128 dwconv output span per group
VLEN = RSTRIP * PITCH + 2   # 16514 v-resident flat length (junk tail slot)
TAPS = [(dy, dx) for dy in (-1, 0, 1) for dx in (-1, 0, 1)]
NPC = RSTRIP * 2     # 128 transpose pos-chunks per core
MASK_NEG = -1.0e4

_CACHE = {}


def _build_program():
    from concourse import bass, mybir, tile, bacc
    from concourse.masks import make_identity

    dt = mybir.dt
    f32, bf = dt.float32, dt.bfloat16
    f32r = dt.float32r
    Copy = mybir.ActivationFunctionType.Copy
    Exp = mybir.ActivationFunctionType.Exp
    Sqrt = mybir.ActivationFunctionType.Sqrt
    AX = mybir.AxisListType.X

    nc = bacc.Bacc(
        "TRN2",
        target_bir_lowering=False,
        debug=False,
        enable_asserts=False,
        num_devices=NCORES,
    )

    def din(name, shape, dty):
        return nc.dram_tensor(name, shape, dty, kind="ExternalInput").ap()

    x_d = din("x_strip", [C, 66 * PITCH], bf)
    xr_d = din("xr_strip", [C, 66 * PITCH], bf)
    wq_d = din("wqT", [C, C], bf)            # [ci, co]
    wkv_d = din("wkvT", [C, 2 * C], bf)      # [ci, co]
    wp_d = din("wprojT", [C, C], bf)         # [ci, co]
    dq128_d = din("dq128", [128, 9 * 128], bf)
    dq64_d = din("dq64", [64, 9 * 64], bf)
    dk128_d = din("dk128", [128, 9 * 128], bf)
    dk64_d = din("dk64", [64, 9 * 64], bf)
    dv64_d = din("dv64", [64, 9 * 64], bf)
    dv128_d = din("dv128", [128, 9 * 128], bf)
    mask_d = din("mask", [C, C], f32)
    temp_d = din("temp_ch", [C, 1], f32)
    out_d = nc.dram_tensor(
        "out_strip", [C, RSTRIP * W], f32, kind="ExternalOutput"
    ).ap()

    RG16 = RG * PITCH  # flat stride of one group's rows in the strip

    with tile.TileContext(nc) as tc:
        with (
            tc.tile_pool(name="const", bufs=1) as cst,
            tc.tile_pool(name="vres", bufs=1) as vp,
            tc.tile_pool(name="p15", bufs=1) as p15,
            tc.tile_pool(name="dram", bufs=1, space="DRAM") as dramp,
        ):
            # ---- constants ----
            wqA = cst.tile([128, C], bf)
            wqB = cst.tile([64, C], bf)
            wkvA = cst.tile([128, 2 * C], bf)
            wkvB = cst.tile([64, 2 * C], bf)
            wpA = cst.tile([128, C], bf)
            wpB = cst.tile([64, C], bf)
            dq128 = cst.tile([128, 9 * 128], bf)
            dq64 = cst.tile([64, 9 * 64], bf)
            dk128 = cst.tile([128, 9 * 128], bf)
            dk64 = cst.tile([64, 9 * 64], bf)
            dv64 = cst.tile([128, 9 * 64], bf)  # data in rows 64:128
            dv128 = cst.tile([128, 9 * 128], bf)
            mask1 = cst.tile([128, C], f32)
            mask2 = cst.tile([64, C], f32)
            temp1 = cst.tile([128, 1], f32)
            temp2 = cst.tile([64, 1], f32)
            ones = cst.tile([128, 1], bf)
            idb = cst.tile([128, 128], bf)
            idf = cst.tile([128, 128], f32)

            nc.sync.dma_start(wqA[:], wq_d[0:128, :])
            nc.sync.dma_start(wqB[:], wq_d[128:192, :])
            nc.sync.dma_start(wkvA[:], wkv_d[0:128, :])
            nc.sync.dma_start(wkvB[:], wkv_d[128:192, :])
            nc.sync.dma_start(wpA[:], wp_d[0:128, :])
            nc.sync.dma_start(wpB[:], wp_d[128:192, :])
            nc.sync.dma_start(dq128[:], dq128_d[:, :])
            nc.sync.dma_start(dq64[:], dq64_d[:, :])
            nc.sync.dma_start(dk128[:], dk128_d[:, :])
            nc.sync.dma_start(dk64[:], dk64_d[:, :])
            nc.sync.dma_start(dv64[64:128, :], dv64_d[:, :])
            nc.sync.dma_start(dv128[:], dv128_d[:, :])
            nc.sync.dma_start(mask1[:], mask_d[0:128, :])
            nc.sync.dma_start(mask2[:], mask_d[128:192, :])
            nc.sync.dma_start(temp1[:], temp_d[0:128, :])
            nc.sync.dma_start(temp2[:], temp_d[128:192, :])
            nc.gpsimd.memset(ones[:], 1.0)
            make_identity(nc, idb[:])
            make_identity(nc, idf[:])

            # ---- v residents (v-ch 0:64 on vX, 64:192 on vY) ----
            vX = vp.tile([64, VLEN], bf)
            vY = vp.tile([128, VLEN], bf)

            with (
                tc.tile_pool(name="xw", bufs=2) as xwp,
                tc.tile_pool(name="pre", bufs=3) as prep,
                tc.tile_pool(name="qkg", bufs=4) as qkgp,
                tc.tile_pool(name="qkT", bufs=3) as qkTp,
                tc.tile_pool(name="sqp", bufs=3) as sqp,
                tc.tile_pool(name="cps", bufs=3, space="PSUM") as cps,
                tc.tile_pool(name="trps", bufs=2, space="PSUM") as trps,
                tc.tile_pool(name="acc", bufs=1, space="PSUM") as accp,
            ):
                S1 = accp.tile([128, C], f32)   # S'[k-ch 0:128, q-ch]
                S2 = accp.tile([64, C], f32)    # S'[k-ch 128:192, q-ch]
                nrm = accp.tile([128, 3], f32)  # packed sum-of-squares

                def run_dwconv(specs, g):
                    # spec: (src, src_row0, nrows, diag, diag_w, dests)
                    # dest: (dst_tile, dst_row0, psum_row0, nrows)
                    for nt in range((DWN + 511) // 512):
                        n0 = 512 * nt
                        nw = min(512, DWN - n0)
                        wb = PITCH + 1 + n0  # operand center base in window
                        for si, (src, sr0, nr, dg, dgw, dests) in \
                                enumerate(specs):
                            psd = cps.tile([128, 512], f32, tag="ps")
                            for ti, (dy, dx) in enumerate(TAPS):
                                dd = dy * PITCH + dx
                                nc.tensor.matmul(
                                    psd[0:nr, :nw],
                                    dg[sr0:sr0 + nr,
                                       ti * dgw:ti * dgw + nr],
                                    src[sr0:sr0 + nr,
                                        wb + dd:wb + dd + nw],
                                    start=(ti == 0), stop=(ti == 8),
                                )
                            for (dtile, dr0, pr0, dnr) in dests:
                                if dtile is vX or dtile is vY:
                                    o0 = g * RG16 + 1 + n0
                                    nc.scalar.activation(
                                        dtile[dr0:dr0 + dnr, o0:o0 + nw],
                                        psd[pr0:pr0 + dnr, :nw], Copy)
                                elif si % 2 == 0:
                                    nc.vector.tensor_copy(
                                        dtile[dr0:dr0 + dnr, n0:n0 + nw],
                                        psd[pr0:pr0 + dnr, :nw])
                                else:
                                    nc.scalar.activation(
                                        dtile[dr0:dr0 + dnr, n0:n0 + nw],
                                        psd[pr0:pr0 + dnr, :nw], Copy)

                ci_count = 0
                for g in range(NG):
                    base_d = g * RG16  # window start in the 66-row strip

                    xa = xwp.tile([128, WIN], bf, tag="xwa")
                    xb = xwp.tile([64, WIN], bf, tag="xwb")
                    nc.sync.dma_start(xa[:], x_d[0:128, base_d:base_d + WIN])
                    nc.sync.dma_start(xb[:], x_d[128:192, base_d:base_d + WIN])

                    qpA = prep.tile([128, WINA], bf, tag="pre")
                    qpB = prep.tile([128, WINA], bf, tag="pre")

                    # conv1x1 q
                    for nt in range((WIN + 511) // 512):
                        n0 = 512 * nt
                        nw = min(512, WIN - n0)
                        ps1 = cps.tile([128, 512], f32, tag="ps")
                        nc.tensor.matmul(ps1[:, :nw], wqA[:, 0:128],
                                         xa[:, n0:n0 + nw],
                                         start=True, stop=False)
                        nc.tensor.matmul(ps1[:, :nw], wqB[:, 0:128],
                                         xb[:, n0:n0 + nw],
                                         start=False, stop=True)
                        nc.scalar.activation(qpA[:, n0:n0 + nw], ps1[:, :nw],
                                             Copy)
                        ps2 = cps.tile([128, 512], f32, tag="ps")
                        nc.tensor.matmul(ps2[0:64, :nw], wqA[:, 128:192],
                                         xa[:, n0:n0 + nw],
                                         start=True, stop=False)
                        nc.tensor.matmul(ps2[0:64, :nw], wqB[:, 128:192],
                                         xb[:, n0:n0 + nw],
                                         start=False, stop=True)
                        nc.vector.tensor_copy(qpB[0:64, n0:n0 + nw],
                                              ps2[0:64, :nw])

                    # dwconv q via diagonal matmuls (frees qpA/qpB early)
                    qgA = qkgp.tile([128, DWN + 32], bf, tag="qkg")
                    qgB = qkgp.tile([128, DWN + 32], bf, tag="qkg")
                    q_specs = [
                        (qpA, 0, 128, dq128, 128, [(qgA, 0, 0, 128)]),
                        (qpB, 0, 64, dq64, 64, [(qgB, 0, 0, 64)]),
                    ]
                    run_dwconv(q_specs, g)

                    xra = xwp.tile([128, WIN], bf, tag="xwa")
                    xrb = xwp.tile([64, WIN], bf, tag="xwb")
                    nc.sync.dma_start(xra[:], xr_d[0:128, base_d:base_d + WIN])
                    nc.sync.dma_start(xrb[:], xr_d[128:192, base_d:base_d + WIN])

                    kvT0 = prep.tile([128, WINA], bf, tag="pre")
                    kvT1 = prep.tile([128, WINA], bf, tag="pre")
                    kvT2 = prep.tile([128, WINA], bf, tag="pre")

                    # conv1x1 kv (M chunks 0:128, 128:256, 256:384)
                    for nt in range((WIN + 511) // 512):
                        n0 = 512 * nt
                        nw = min(512, WIN - n0)
                        for mi, dst in enumerate((kvT0, kvT1, kvT2)):
                            m0 = 128 * mi
                            ps1 = cps.tile([128, 512], f32, tag="ps")
                            nc.tensor.matmul(ps1[:, :nw],
                                             wkvA[:, m0:m0 + 128],
                                             xra[:, n0:n0 + nw],
                                             start=True, stop=False)
                            nc.tensor.matmul(ps1[:, :nw],
                                             wkvB[:, m0:m0 + 128],
                                             xrb[:, n0:n0 + nw],
                                             start=False, stop=True)
                            if mi % 2 == 0:
                                nc.vector.tensor_copy(dst[:, n0:n0 + nw],
                                                      ps1[:, :nw])
                            else:
                                nc.scalar.activation(dst[:, n0:n0 + nw],
                                                     ps1[:, :nw], Copy)

                    # dwconv k, v via diagonal matmuls
                    kgA = qkgp.tile([128, DWN + 32], bf, tag="qkg")
                    kgB = qkgp.tile([128, DWN + 32], bf, tag="qkg")
                    kv_specs = [
                        (kvT0, 0, 128, dk128, 128, [(kgA, 0, 0, 128)]),
                        (kvT1, 0, 64, dk64, 64, [(kgB, 0, 0, 64)]),
                        (kvT1, 64, 64, dv64, 64, [(vX, 0, 0, 64)]),
                        (kvT2, 0, 128, dv128, 128, [(vY, 0, 0, 128)]),
                    ]
                    run_dwconv(kv_specs, g)

                    # transposes + gram + norms per 128-pos chunk
                    for ry in range(RG):
                        for j in range(2):
                            pc = ry * PITCH + 128 * j
                            trp = trps.tile([128, 384], bf, tag="trp")
                            nc.tensor.transpose(trp[:, 0:128],
                                                qgA[:, pc:pc + 128], idb[:])
                            nc.tensor.transpose(trp[:, 128:192],
                                                qgB[0:64, pc:pc + 128],
                                                idb[0:64, 0:64])
                            nc.tensor.transpose(trp[:, 192:320],
                                                kgA[:, pc:pc + 128], idb[:])
                            nc.tensor.transpose(trp[:, 320:384],
                                                kgB[0:64, pc:pc + 128],
                                                idb[0:64, 0:64])
                            qkT = qkTp.tile([128, 384], bf, tag="qkT")
                            nc.vector.tensor_copy(qkT[:], trp[:])
                            sq = sqp.tile([128, 384], bf, tag="sq")
                            nc.vector.tensor_mul(sq[:], qkT[:], qkT[:])

                            st = ci_count == 0
                            sp = ci_count == (NG * RG * 2 - 1)
                            nc.tensor.matmul(S1[:, :], qkT[:, 192:320],
                                             qkT[:, 0:192],
                                             start=st, stop=sp)
                            nc.tensor.matmul(S2[:, :], qkT[:, 320:384],
                                             qkT[:, 0:192],
                                             start=st, stop=sp)
                            nc.tensor.matmul(nrm[:, 0:1], sq[:, 0:128],
                                             ones[:], start=st, stop=sp)
                            nc.tensor.matmul(nrm[:, 1:2], sq[:, 128:256],
                                             ones[:], start=st, stop=sp)
                            nc.tensor.matmul(nrm[:, 2:3], sq[:, 256:384],
                                             ones[:], start=st, stop=sp)
                            ci_count += 1

                # evacuate accumulators before psum pools close
                Ssb1 = p15.tile([128, C], f32)
                Ssb2 = p15.tile([64, C], f32)
                nrmsb = p15.tile([128, 3], f32)
                nc.vector.tensor_copy(Ssb1[:], S1[:])
                nc.vector.tensor_copy(Ssb2[:], S2[:])
                nc.vector.tensor_copy(nrmsb[:], nrm[:])

            # ---- phase 1.5: allreduce + softmax + attnT ----
            db_in = dramp.tile([C, 194], f32)
            db_out = dramp.tile([C, 194], f32)
            nc.gpsimd.dma_start(db_in[0:128, 0:192], Ssb1[:])
            nc.gpsimd.dma_start(db_in[128:192, 0:192], Ssb2[:])
            nc.gpsimd.dma_start(db_in[0:128, 192:193], nrmsb[:, 0:1])
            nc.gpsimd.dma_start(db_in[128:192, 192:193], nrmsb[0:64, 1:2])
            nc.gpsimd.dma_start(db_in[0:64, 193:194], nrmsb[64:128, 1:2])
            nc.gpsimd.dma_start(db_in[64:192, 193:194], nrmsb[:, 2:3])
            nc.gpsimd.collective_compute(
                "AllReduce",
                mybir.AluOpType.add,
                replica_groups=[[0, 1, 2, 3], [4, 5, 6, 7]],
                ins=[db_in[:].opt()],
                outs=[db_out[:].opt()],
            )
            Sr1 = p15.tile([128, 194], f32)
            Sr2 = p15.tile([64, 194], f32)
            nc.gpsimd.dma_start(Sr1[:], db_out[0:128, :])
            nc.gpsimd.dma_start(Sr2[:], db_out[128:192, :])

            # inverse norms
            iq1 = p15.tile([128, 1], f32)
            iq2 = p15.tile([64, 1], f32)
            ik1 = p15.tile([128, 1], f32)
            ik2 = p15.tile([64, 1], f32)
            tq1 = p15.tile([128, 1], f32)
            tq2 = p15.tile([64, 1], f32)
            nc.scalar.activation(tq1[:], Sr1[:, 192:193], Sqrt)
            nc.vector.reciprocal(iq1[:], tq1[:])
            nc.scalar.activation(tq2[:], Sr2[:, 192:193], Sqrt)
            nc.vector.reciprocal(iq2[:], tq2[:])
            nc.scalar.activation(tq1[:], Sr1[:, 193:194], Sqrt)
            nc.vector.reciprocal(ik1[:], tq1[:])
            nc.scalar.activation(tq2[:], Sr2[:, 193:194], Sqrt)
            nc.vector.reciprocal(ik2[:], tq2[:])

            # scale rows by 1/|k| then transpose to [q-ch, k-ch]
            SsA1 = p15.tile([128, C], f32)
            SsA2 = p15.tile([64, C], f32)
            nc.vector.tensor_scalar_mul(SsA1[:], Sr1[:, 0:192], ik1[:])
            nc.vector.tensor_scalar_mul(SsA2[:], Sr2[:, 0:192], ik2[:])

            rs1 = p15.tile([128, 1], f32)
            rs2 = p15.tile([64, 1], f32)
            nc.vector.tensor_mul(rs1[:], temp1[:], iq1[:])
            nc.vector.tensor_mul(rs2[:], temp2[:], iq2[:])

            with tc.tile_pool(name="p15ps", bufs=2, space="PSUM") as pp:
                trS1 = pp.tile([128, C], f32)
                trS2 = pp.tile([64, C], f32)
                nc.tensor.transpose(trS1[:, 0:128], SsA1[:, 0:128], idf[:])
                nc.tensor.transpose(trS1[:, 128:192], SsA2[:, 0:128],
                                    idf[0:64, 0:64])
                nc.tensor.transpose(trS2[0:64, 0:128], SsA1[:, 128:192],
                                    idf[:])
                nc.tensor.transpose(trS2[0:64, 128:192], SsA2[:, 128:192],
                                    idf[0:64, 0:64])
                Sm1 = p15.tile([128, C], f32)
                Sm2 = p15.tile([64, C], f32)
                nc.vector.tensor_scalar_mul(Sm1[:], trS1[:], rs1[:])
                nc.vector.tensor_scalar_mul(Sm2[:], trS2[:], rs2[:])
                nc.vector.tensor_add(Sm1[:], Sm1[:], mask1[:])
                nc.vector.tensor_add(Sm2[:], Sm2[:], mask2[:])

                # softmax over free dim
                mx1 = p15.tile([128, 1], f32)
                mx2 = p15.tile([64, 1], f32)
                nc.vector.reduce_max(mx1[:], Sm1[:], axis=AX)
                nc.vector.reduce_max(mx2[:], Sm2[:], axis=AX)
                nc.vector.tensor_scalar_mul(mx1[:], mx1[:], -1.0)
                nc.vector.tensor_scalar_mul(mx2[:], mx2[:], -1.0)
                E1 = p15.tile([128, C], f32)
                E2 = p15.tile([64, C], f32)
                es1 = p15.tile([128, 1], f32)
                es2 = p15.tile([64, 1], f32)
                nc.scalar.activation(E1[:], Sm1[:], Exp, bias=mx1[:],
                                     scale=1.0, accum_out=es1[:])
                nc.scalar.activation(E2[:], Sm2[:], Exp, bias=mx2[:],
                                     scale=1.0, accum_out=es2[:])
                nc.vector.reciprocal(es1[:], es1[:])
                nc.vector.reciprocal(es2[:], es2[:])
                at1 = p15.tile([128, C], f32)
                at2 = p15.tile([64, C], f32)
                nc.vector.tensor_scalar_mul(at1[:], E1[:], es1[:])
                nc.vector.tensor_scalar_mul(at2[:], E2[:], es2[:])

                # attnT [d, c]: attnTx holds d 0:64, attnTy holds d 64:192
                trAx = pp.tile([64, C], f32)
                trAy = pp.tile([128, C], f32)
                nc.tensor.transpose(trAx[0:64, 0:128], at1[:, 0:64], idf[:])
                nc.tensor.transpose(trAx[0:64, 128:192], at2[:, 0:64],
                                    idf[0:64, 0:64])
                nc.tensor.transpose(trAy[:, 0:128], at1[:, 64:192], idf[:])
                nc.tensor.transpose(trAy[:, 128:192], at2[:, 64:192],
                                    idf[0:64, 0:64])
                attnTx = p15.tile([64, C], bf)
                attnTy = p15.tile([128, C], bf)
                nc.vector.tensor_copy(attnTx[:], trAx[:])
                nc.vector.tensor_copy(attnTy[:], trAy[:])

            # ---- phase 2: out = proj(attn @ v) ----
            with (
                tc.tile_pool(name="p2", bufs=4) as p2p,
                tc.tile_pool(name="p2ps", bufs=8, space="PSUM") as p2ps,
            ):
                for t in range(RSTRIP // 2):
                    y0 = 2 * t
                    vb = y0 * PITCH + 1
                    # moving v: [part, 2 rows, 256] strided by PITCH
                    vAx = vX[0:64, vb:vb + 2 * PITCH].rearrange(
                        "p (r w) -> p r w", r=2)[:, :, 0:W]
                    vAy = vY[:, vb:vb + 2 * PITCH].rearrange(
                        "p (r w) -> p r w", r=2)[:, :, 0:W]
                    psA = p2ps.tile([128, 512], f32, tag="p2")
                    psB = p2ps.tile([128, 512], f32, tag="p2")
                    nc.tensor.matmul(psA[:, :], attnTx[0:64, 0:128], vAx,
                                     start=True, stop=False)
                    nc.tensor.matmul(psA[:, :], attnTy[:, 0:128], vAy,
                                     start=False, stop=True)
                    nc.tensor.matmul(psB[0:64, :], attnTx[0:64, 128:192],
                                     vAx, start=True, stop=False)
                    nc.tensor.matmul(psB[0:64, :], attnTy[:, 128:192], vAy,
                                     start=False, stop=True)
                    oh1 = p2p.tile([128, 512], bf, tag="oh")
                    oh2 = p2p.tile([128, 512], bf, tag="oh")
                    nc.vector.tensor_copy(oh1[:], psA[:])
                    nc.scalar.activation(oh2[0:64, :], psB[0:64, :], Copy)

                    psC = p2ps.tile([128, 512], f32, tag="p2")
                    psD = p2ps.tile([128, 512], f32, tag="p2")
                    nc.tensor.matmul(psC[:, :], wpA[:, 0:128], oh1[:],
                                     start=True, stop=False)
                    nc.tensor.matmul(psC[:, :], wpB[:, 0:128],
                                     oh2[0:64, :], start=False, stop=True)
                    nc.tensor.matmul(psD[0:64, :], wpA[:, 128:192],
                                     oh1[:], start=True, stop=False)
                    nc.tensor.matmul(psD[0:64, :], wpB[:, 128:192],
                                     oh2[0:64, :], start=False, stop=True)
                    o1 = p2p.tile([128, 512], f32, tag="o")
                    o2 = p2p.tile([128, 512], f32, tag="o")
                    nc.vector.tensor_copy(o1[:], psC[:])
                    nc.scalar.activation(o2[0:64, :], psD[0:64, :], Copy)
                    ob = y0 * W
                    nc.sync.dma_start(out_d[0:128, ob:ob + 512], o1[:])
                    nc.sync.dma_start(out_d[128:192, ob:ob + 512],
                                      o2[0:64, :])

    nc.compile()
    return nc


def _diag_blocks(w9, nch):
    """w9: [nch, 9] tap weights -> [nch, 9*nch] with diag blocks per tap."""
    out = np.zeros((nch, 9 * nch), np.float32)
    idx = np.arange(nch)
    for t in range(9):
        out[idx, t * nch + idx] = w9[:, t]
    return out.astype(BF)


def _prep_consts(wq, wq_dw, wkv, wkv_dw, wproj, temperature):
    w9q = wq_dw.reshape(C, 9).astype(np.float32)
    w9kv = wkv_dw.reshape(2 * C, 9).astype(np.float32)
    mask = np.full((C, C), MASK_NEG, np.float32)
    for h in range(HEADS):
        mask[h * CH:(h + 1) * CH, h * CH:(h + 1) * CH] = 0.0
    consts = {
        "wqT": np.ascontiguousarray(wq.T).astype(BF),
        "wkvT": np.ascontiguousarray(wkv.T).astype(BF),
        "wprojT": np.ascontiguousarray(wproj.T).astype(BF),
        "dq128": _diag_blocks(w9q[0:128], 128),
        "dq64": _diag_blocks(w9q[128:192], 64),
        "dk128": _diag_blocks(w9kv[0:128], 128),
        "dk64": _diag_blocks(w9kv[128:192], 64),
        "dv64": _diag_blocks(w9kv[192:256], 64),
        "dv128": _diag_blocks(w9kv[256:384], 128),
        "mask": mask,
        "temp_ch": np.repeat(
            np.asarray(temperature, np.float32).reshape(HEADS), CH
        ).reshape(C, 1).astype(np.float32),
    }
    return consts


def kernel(x, x_ref, wq, wq_dw, wkv, wkv_dw, wproj, temperature):
    from concourse.bass_utils import run_bass_kernel_spmd

    if "nc" not in _CACHE:
        _CACHE["nc"] = _build_program()
    nc = _CACHE["nc"]

    x = np.asarray(x, np.float32)
    x_ref = np.asarray(x_ref, np.float32)
    consts = _prep_consts(
        np.asarray(wq, np.float32), np.asarray(wq_dw, np.float32),
        np.asarray(wkv, np.float32), np.asarray(wkv_dw, np.float32),
        np.asarray(wproj, np.float32), np.asarray(temperature, np.float32),
    )

    xp = np.pad(x, ((0, 0), (0, 0), (1, 1), (1, 1))).astype(BF)
    xrp = np.pad(x_ref, ((0, 0), (0, 0), (1, 1), (1, 1))).astype(BF)

    in_maps = []
    for core in range(NCORES):
        b, r = core // 4, core % 4
        m = dict(consts)
        m["x_strip"] = np.ascontiguousarray(
            xp[b, :, 64 * r:64 * r + 66, :]).reshape(C, 66 * PITCH)
        m["xr_strip"] = np.ascontiguousarray(
            xrp[b, :, 64 * r:64 * r + 66, :]).reshape(C, 66 * PITCH)
        in_maps.append(m)

    res = run_bass_kernel_spmd(nc, in_maps, core_ids=list(range(NCORES)))
    out = np.zeros((B, C, H, W), np.float32)
    for core in range(NCORES):
        b, r = core // 4, core % 4
        strip = np.asarray(res.results[core]["out_strip"], np.float32)
        out[b, :, 64 * r:64 * r + 64, :] = strip.reshape(C, 64, W)
    return out


# revision 18
# speedup vs baseline: 673.7958x; 673.7958x over previous
"""Trainium2 Bass kernel for nn_CrossAttention (channel attention / XCA-style).

Sharding: 8 cores = 2 batches x 4 row-strips of 64 rows. Each core computes
its strip end-to-end; the attention score matrix S = k q^T and the q/k row
norms are partial-summed per core and AllReduced across the 4 cores of the
same batch (replica groups [[0..3],[4..7]]), after which every core finishes
softmax + attn@v + projection for its own strip.

Layouts inside a core (channels on partitions, 192 = 128+64 chunks):
  conv1x1 (bf16 matmul)  ->  pre [ch, (rows+2)*258] padded windows
  dwconv3x3 = 9 diagonal-stationary matmuls with free-dim-shifted operands
  q,k -> PE-transposed per 128-pos chunk -> gram matmuls accumulate S' = k q^T
  norms via squared-transpose x ones matmuls
  softmax on S'' (masked to per-head 24x24 blocks), attn@v + proj in fp32r.
"""

import sys
import numpy as np

if "/opt/trn_rl_repo" not in sys.path:
    sys.path.insert(0, "/opt/trn_rl_repo")

import ml_dtypes

BF = ml_dtypes.bfloat16

B = 2
C = 192
HEADS = 8
CH = C // HEADS  # 24
H = W = 256
PITCH = W + 2  # 258
NCORES = 8
RSTRIP = 64          # rows per core
RG = 16              # rows per group
NG = RSTRIP // RG    # 4
WINR = RG + 2        # 18
WIN = WINR * PITCH   # 4644
WINA = WIN + 2       # pre-window alloc (dwconv operand slop)
DWN = RG * PITCH     # 4128 dwconv output span per group
VLEN = RSTRIP * PITCH + 2   # 16514 v-resident flat length (junk tail slot)
TAPS = [(dy, dx) for dy in (-1, 0, 1) for dx in (-1, 0, 1)]
NPC = RSTRIP * 2     # 128 transpose pos-chunks per core
MASK_NEG = -1.0e4

_CACHE = {}


def _build_program():
    from concourse import bass, mybir, tile, bacc
    from concourse.masks import make_identity

    dt = mybir.dt
    f32, bf = dt.float32, dt.bfloat16
    f32r = dt.float32r
    Copy = mybir.ActivationFunctionType.Copy
    Exp = mybir.ActivationFunctionType.Exp
    Sqrt = mybir.ActivationFunctionType.Sqrt
    AX = mybir.AxisListType.X

    nc = bacc.Bacc(
        "TRN2",
        target_bir_lowering=False,
        debug=False,
        enable_asserts=False,
        num_devices=NCORES,
    )

    def din(name, shape, dty):
        return nc.dram_tensor(name, shape, dty, kind="ExternalInput").ap()

    x_d = din("x_strip", [C, 66 * PITCH], bf)
    xr_d = din("xr_strip", [C, 66 * PITCH], bf)
    wq_d = din("wqT", [C, C], bf)            # [ci, co]
    wkv_d = din("wkvT", [C, 2 * C], bf)      # [ci, co]
    wp_d = din("wprojT", [C, C], bf)         # [ci, co]
    dq128_d = din("dq128", [128, 9 * 128], bf)
    dq64_d = din("dq64", [64, 9 * 64], bf)
    dk128_d = din("dk128", [128, 9 * 128], bf)
    dk64_d = din("dk64", [64, 9 * 64], bf)
    dv64_d = din("dv64", [64, 9 * 64], bf)
    dv128_d = din("dv128", [128, 9 * 128], bf)
    mask_d = din("mask", [C, C], f32)
    temp_d = din("temp_ch", [C, 1], f32)
    out_d = nc.dram_tensor(
        "out_strip", [C, RSTRIP * W], f32, kind="ExternalOutput"
    ).ap()

    RG16 = RG * PITCH  # flat stride of one group's rows in the strip

    with tile.TileContext(nc) as tc:
        with (
            tc.tile_pool(name="const", bufs=1) as cst,
            tc.tile_pool(name="vres", bufs=1) as vp,
            tc.tile_pool(name="p15", bufs=1) as p15,
            tc.tile_pool(name="dram", bufs=1, space="DRAM") as dramp,
        ):
            # ---- constants ----
            wqA = cst.tile([128, C], bf)
            wqB = cst.tile([64, C], bf)
            wkvA = cst.tile([128, 2 * C], bf)
            wkvB = cst.tile([64, 2 * C], bf)
            wpA = cst.tile([128, C], bf)
            wpB = cst.tile([64, C], bf)
            dq128 = cst.tile([128, 9 * 128], bf)
            dq64 = cst.tile([64, 9 * 64], bf)
            dk128 = cst.tile([128, 9 * 128], bf)
            dk64 = cst.tile([64, 9 * 64], bf)
            dv64 = cst.tile([128, 9 * 64], bf)  # data in rows 64:128
            dv128 = cst.tile([128, 9 * 128], bf)
            mask1 = cst.tile([128, C], f32)
            mask2 = cst.tile([64, C], f32)
            temp1 = cst.tile([128, 1], f32)
            temp2 = cst.tile([64, 1], f32)
            ones = cst.tile([128, 1], bf)
            idb = cst.tile([128, 128], bf)
            idf = cst.tile([128, 128], f32)

            nc.sync.dma_start(wqA[:], wq_d[0:128, :])
            nc.sync.dma_start(wqB[:], wq_d[128:192, :])
            nc.sync.dma_start(wkvA[:], wkv_d[0:128, :])
            nc.sync.dma_start(wkvB[:], wkv_d[128:192, :])
            nc.sync.dma_start(wpA[:], wp_d[0:128, :])
            nc.sync.dma_start(wpB[:], wp_d[128:192, :])
            nc.sync.dma_start(dq128[:], dq128_d[:, :])
            nc.sync.dma_start(dq64[:], dq64_d[:, :])
            nc.sync.dma_start(dk128[:], dk128_d[:, :])
            nc.sync.dma_start(dk64[:], dk64_d[:, :])
            nc.sync.dma_start(dv64[64:128, :], dv64_d[:, :])
            nc.sync.dma_start(dv128[:], dv128_d[:, :])
            nc.sync.dma_start(mask1[:], mask_d[0:128, :])
            nc.sync.dma_start(mask2[:], mask_d[128:192, :])
            nc.sync.dma_start(temp1[:], temp_d[0:128, :])
            nc.sync.dma_start(temp2[:], temp_d[128:192, :])
            nc.gpsimd.memset(ones[:], 1.0)
            make_identity(nc, idb[:])
            make_identity(nc, idf[:])

            # ---- v residents (v-ch 0:64 on vX, 64:192 on vY) ----
            vX = vp.tile([64, VLEN], bf)
            vY = vp.tile([128, VLEN], bf)

            with (
                tc.tile_pool(name="xw", bufs=2) as xwp,
                tc.tile_pool(name="pre", bufs=3) as prep,
                tc.tile_pool(name="qkg", bufs=4) as qkgp,
                tc.tile_pool(name="qkT", bufs=3) as qkTp,
                tc.tile_pool(name="sqp", bufs=3) as sqp,
                tc.tile_pool(name="cps", bufs=3, space="PSUM") as cps,
                tc.tile_pool(name="trps", bufs=2, space="PSUM") as trps,
                tc.tile_pool(name="acc", bufs=1, space="PSUM") as accp,
            ):
                S1 = accp.tile([128, C], f32)   # S'[k-ch 0:128, q-ch]
                S2 = accp.tile([64, C], f32)    # S'[k-ch 128:192, q-ch]
                nrm = accp.tile([128, 3], f32)  # packed sum-of-squares

                def run_dwconv(specs, g):
                    # spec: (src, src_row0, nrows, diag, diag_w, dests)
                    # dest: (dst_tile, dst_row0, psum_row0, nrows)
                    for nt in range((DWN + 511) // 512):
                        n0 = 512 * nt
                        nw = min(512, DWN - n0)
                        wb = PITCH + 1 + n0  # operand center base in window
                        for si, (src, sr0, nr, dg, dgw, dests) in \
                                enumerate(specs):
                            psd = cps.tile([128, 512], f32, tag="ps")
                            for ti, (dy, dx) in enumerate(TAPS):
                                dd = dy * PITCH + dx
                                nc.tensor.matmul(
                                    psd[0:nr, :nw],
                                    dg[sr0:sr0 + nr,
                                       ti * dgw:ti * dgw + nr],
                                    src[sr0:sr0 + nr,
                                        wb + dd:wb + dd + nw],
                                    start=(ti == 0), stop=(ti == 8),
                                )
                            for (dtile, dr0, pr0, dnr) in dests:
                                if dtile is vX or dtile is vY:
                                    o0 = g * RG16 + 1 + n0
                                    nc.scalar.activation(
                                        dtile[dr0:dr0 + dnr, o0:o0 + nw],
                                        psd[pr0:pr0 + dnr, :nw], Copy)
                                elif si % 2 == 0:
                                    nc.vector.tensor_copy(
                                        dtile[dr0:dr0 + dnr, n0:n0 + nw],
                                        psd[pr0:pr0 + dnr, :nw])
                                else:
                                    nc.scalar.activation(
                                        dtile[dr0:dr0 + dnr, n0:n0 + nw],
                                        psd[pr0:pr0 + dnr, :nw], Copy)

                ci_count = 0
                for g in range(NG):
                    base_d = g * RG16  # window start in the 66-row strip

                    xa = xwp.tile([128, WIN], bf, tag="xwa")
                    xb = xwp.tile([64, WIN], bf, tag="xwb")
                    nc.sync.dma_start(xa[:], x_d[0:128, base_d:base_d + WIN])
                    nc.sync.dma_start(xb[:], x_d[128:192, base_d:base_d + WIN])

                    qpA = prep.tile([128, WINA], bf, tag="pre")
                    qpB = prep.tile([128, WINA], bf, tag="pre")

                    # conv1x1 q
                    for nt in range((WIN + 511) // 512):
                        n0 = 512 * nt
                        nw = min(512, WIN - n0)
                        ps1 = cps.tile([128, 512], f32, tag="ps")
                        nc.tensor.matmul(ps1[:, :nw], wqA[:, 0:128],
                                         xa[:, n0:n0 + nw],
                                         start=True, stop=False)
                        nc.tensor.matmul(ps1[:, :nw], wqB[:, 0:128],
                                         xb[:, n0:n0 + nw],
                                         start=False, stop=True)
                        nc.scalar.activation(qpA[:, n0:n0 + nw], ps1[:, :nw],
                                             Copy)
                        ps2 = cps.tile([128, 512], f32, tag="ps")
                        nc.tensor.matmul(ps2[0:64, :nw], wqA[:, 128:192],
                                         xa[:, n0:n0 + nw],
                                         start=True, stop=False)
                        nc.tensor.matmul(ps2[0:64, :nw], wqB[:, 128:192],
                                         xb[:, n0:n0 + nw],
                                         start=False, stop=True)
                        nc.vector.tensor_copy(qpB[0:64, n0:n0 + nw],
                                              ps2[0:64, :nw])

                    # dwconv q via diagonal matmuls (frees qpA/qpB early)
                    qgA = qkgp.tile([128, DWN + 32], bf, tag="qkg")
                    qgB = qkgp.tile([128, DWN + 32], bf, tag="qkg")
                    q_specs = [
                        (qpA, 0, 128, dq128, 128, [(qgA, 0, 0, 128)]),
                        (qpB, 0, 64, dq64, 64, [(qgB, 0, 0, 64)]),
                    ]
                    run_dwconv(q_specs, g)

                    xra = xwp.tile([128, WIN], bf, tag="xwa")
                    xrb = xwp.tile([64, WIN], bf, tag="xwb")
                    nc.sync.dma_start(xra[:], xr_d[0:128, base_d:base_d + WIN])
                    nc.sync.dma_start(xrb[:], xr_d[128:192, base_d:base_d + WIN])

                    kvT0 = prep.tile([128, WINA], bf, tag="pre")
                    kvT1 = prep.tile([128, WINA], bf, tag="pre")
                    kvT2 = prep.tile([128, WINA], bf, tag="pre")

                    # conv1x1 kv (M chunks 0:128, 128:256, 256:384)
                    for nt in range((WIN + 511) // 512):
                        n0 = 512 * nt
                        nw = min(512, WIN - n0)
                        for mi, dst in enumerate((kvT0, kvT1, kvT2)):
                            m0 = 128 * mi
                            ps1 = cps.tile([128, 512], f32, tag="ps")
                            nc.tensor.matmul(ps1[:, :nw],
                                             wkvA[:, m0:m0 + 128],
                                             xra[:, n0:n0 + nw],
                                             start=True, stop=False)
                            nc.tensor.matmul(ps1[:, :nw],
                                             wkvB[:, m0:m0 + 128],
                                             xrb[:, n0:n0 + nw],
                                             start=False, stop=True)
                            if mi % 2 == 0:
                                nc.vector.tensor_copy(dst[:, n0:n0 + nw],
                                                      ps1[:, :nw])
                            else:
                                nc.scalar.activation(dst[:, n0:n0 + nw],
                                                     ps1[:, :nw], Copy)

                    # dwconv k, v via diagonal matmuls
                    kgA = qkgp.tile([128, DWN + 32], bf, tag="qkg")
                    kgB = qkgp.tile([128, DWN + 32], bf, tag="qkg")
                    kv_specs = [
                        (kvT0, 0, 128, dk128, 128, [(kgA, 0, 0, 128)]),
                        (kvT1, 0, 64, dk64, 64, [(kgB, 0, 0, 64)]),
                        (kvT1, 64, 64, dv64, 64, [(vX, 0, 0, 64)]),
                        (kvT2, 0, 128, dv128, 128, [(vY, 0, 0, 128)]),
                    ]
                    run_dwconv(kv_specs, g)

                    # transposes + gram + norms per 128-pos chunk
                    for ry in range(RG):
                        for j in range(2):
                            pc = ry * PITCH + 128 * j
                            trp = trps.tile([128, 384], bf, tag="trp")
                            nc.tensor.transpose(trp[:, 0:128],
                                                qgA[:, pc:pc + 128], idb[:])
                            nc.tensor.transpose(trp[:, 128:192],
                                                qgB[0:64, pc:pc + 128],
                                                idb[0:64, 0:64])
                            nc.tensor.transpose(trp[:, 192:320],
                                                kgA[:, pc:pc + 128], idb[:])
                            nc.tensor.transpose(trp[:, 320:384],
                                                kgB[0:64, pc:pc + 128],
                                                idb[0:64, 0:64])
                            qkT = qkTp.tile([128, 384], bf, tag="qkT")
                            nc.vector.tensor_copy(qkT[:], trp[:])
                            sq = sqp.tile([128, 384], bf, tag="sq")
                            nc.vector.tensor_mul(sq[:], qkT[:], qkT[:])

                            st = ci_count == 0
                            sp = ci_count == (NG * RG * 2 - 1)
                            nc.tensor.matmul(S1[:, :], qkT[:, 192:320],
                                             qkT[:, 0:192],
                                             start=st, stop=sp)
                            nc.tensor.matmul(S2[:, :], qkT[:, 320:384],
                                             qkT[:, 0:192],
                                             start=st, stop=sp)
                            nc.tensor.matmul(nrm[:, 0:1], sq[:, 0:128],
                                             ones[:], start=st, stop=sp)
                            nc.tensor.matmul(nrm[:, 1:2], sq[:, 128:256],
                                             ones[:], start=st, stop=sp)
                            nc.tensor.matmul(nrm[:, 2:3], sq[:, 256:384],
                                             ones[:], start=st, stop=sp)
                            ci_count += 1

                # evacuate accumulators before psum pools close
                Ssb1 = p15.tile([128, C], f32)
                Ssb2 = p15.tile([64, C], f32)
                nrmsb = p15.tile([128, 3], f32)
                nc.vector.tensor_copy(Ssb1[:], S1[:])
                nc.vector.tensor_copy(Ssb2[:], S2[:])
                nc.vector.tensor_copy(nrmsb[:], nrm[:])

            # ---- phase 1.5: allreduce + softmax + attnT ----
            db_in = dramp.tile([C, 194], f32)
            db_out = dramp.tile([C, 194], f32)
            nc.gpsimd.dma_start(db_in[0:128, 0:192], Ssb1[:])
            nc.gpsimd.dma_start(db_in[128:192, 0:192], Ssb2[:])
            nc.gpsimd.dma_start(db_in[0:128, 192:193], nrmsb[:, 0:1])
            nc.gpsimd.dma_start(db_in[128:192, 192:193], nrmsb[0:64, 1:2])
            nc.gpsimd.dma_start(db_in[0:64, 193:194], nrmsb[64:128, 1:2])
            nc.gpsimd.dma_start(db_in[64:192, 193:194], nrmsb[:, 2:3])
            nc.gpsimd.collective_compute(
                "AllReduce",
                mybir.AluOpType.add,
                replica_groups=[[0, 1, 2, 3], [4, 5, 6, 7]],
                ins=[db_in[:].opt()],
                outs=[db_out[:].opt()],
            )
            Sr1 = p15.tile([128, 194], f32)
            Sr2 = p15.tile([64, 194], f32)
            nc.gpsimd.dma_start(Sr1[:], db_out[0:128, :])
            nc.gpsimd.dma_start(Sr2[:], db_out[128:192, :])

            # inverse norms
            iq1 = p15.tile([128, 1], f32)
            iq2 = p15.tile([64, 1], f32)
            ik1 = p15.tile([128, 1], f32)
            ik2 = p15.tile([64, 1], f32)
            tq1 = p15.tile([128, 1], f32)
            tq2 = p15.tile([64, 1], f32)
            nc.scalar.activation(tq1[:], Sr1[:, 192:193], Sqrt)
            nc.vector.reciprocal(iq1[:], tq1[:])
            nc.scalar.activation(tq2[:], Sr2[:, 192:193], Sqrt)
            nc.vector.reciprocal(iq2[:], tq2[:])
            nc.scalar.activation(tq1[:], Sr1[:, 193:194], Sqrt)
            nc.vector.reciprocal(ik1[:], tq1[:])
            nc.scalar.activation(tq2[:], Sr2[:, 193:194], Sqrt)
            nc.vector.reciprocal(ik2[:], tq2[:])

            # scale rows by 1/|k| then transpose to [q-ch, k-ch]
            SsA1 = p15.tile([128, C], f32)
            SsA2 = p15.tile([64, C], f32)
            nc.vector.tensor_scalar_mul(SsA1[:], Sr1[:, 0:192], ik1[:])
            nc.vector.tensor_scalar_mul(SsA2[:], Sr2[:, 0:192], ik2[:])

            rs1 = p15.tile([128, 1], f32)
            rs2 = p15.tile([64, 1], f32)
            nc.vector.tensor_mul(rs1[:], temp1[:], iq1[:])
            nc.vector.tensor_mul(rs2[:], temp2[:], iq2[:])

            with tc.tile_pool(name="p15ps", bufs=2, space="PSUM") as pp:
                trS1 = pp.tile([128, C], f32)
                trS2 = pp.tile([64, C], f32)
                nc.tensor.transpose(trS1[:, 0:128], SsA1[:, 0:128], idf[:])
                nc.tensor.transpose(trS1[:, 128:192], SsA2[:, 0:128],
                                    idf[0:64, 0:64])
                nc.tensor.transpose(trS2[0:64, 0:128], SsA1[:, 128:192],
                                    idf[:])
                nc.tensor.transpose(trS2[0:64, 128:192], SsA2[:, 128:192],
                                    idf[0:64, 0:64])
                Sm1 = p15.tile([128, C], f32)
                Sm2 = p15.tile([64, C], f32)
                nc.vector.tensor_scalar_mul(Sm1[:], trS1[:], rs1[:])
                nc.vector.tensor_scalar_mul(Sm2[:], trS2[:], rs2[:])
                nc.vector.tensor_add(Sm1[:], Sm1[:], mask1[:])
                nc.vector.tensor_add(Sm2[:], Sm2[:], mask2[:])

                # softmax over free dim
                mx1 = p15.tile([128, 1], f32)
                mx2 = p15.tile([64, 1], f32)
                nc.vector.reduce_max(mx1[:], Sm1[:], axis=AX)
                nc.vector.reduce_max(mx2[:], Sm2[:], axis=AX)
                nc.vector.tensor_scalar_mul(mx1[:], mx1[:], -1.0)
                nc.vector.tensor_scalar_mul(mx2[:], mx2[:], -1.0)
                E1 = p15.tile([128, C], f32)
                E2 = p15.tile([64, C], f32)
                es1 = p15.tile([128, 1], f32)
                es2 = p15.tile([64, 1], f32)
                nc.scalar.activation(E1[:], Sm1[:], Exp, bias=mx1[:],
                                     scale=1.0, accum_out=es1[:])
                nc.scalar.activation(E2[:], Sm2[:], Exp, bias=mx2[:],
                                     scale=1.0, accum_out=es2[:])
                nc.vector.reciprocal(es1[:], es1[:])
                nc.vector.reciprocal(es2[:], es2[:])
                at1 = p15.tile([128, C], f32)
                at2 = p15.tile([64, C], f32)
                nc.vector.tensor_scalar_mul(at1[:], E1[:], es1[:])
                nc.vector.tensor_scalar_mul(at2[:], E2[:], es2[:])

                # attnT [d, c]: attnTx holds d 0:64, attnTy holds d 64:192
                trAx = pp.tile([64, C], f32)
                trAy = pp.tile([128, C], f32)
                nc.tensor.transpose(trAx[0:64, 0:128], at1[:, 0:64], idf[:])
                nc.tensor.transpose(trAx[0:64, 128:192], at2[:, 0:64],
                                    idf[0:64, 0:64])
                nc.tensor.transpose(trAy[:, 0:128], at1[:, 64:192], idf[:])
                nc.tensor.transpose(trAy[:, 128:192], at2[:, 64:192],
                                    idf[0:64, 0:64])
                attnTx = p15.tile([64, C], bf)
                attnTy = p15.tile([128, C], bf)
                nc.vector.tensor_copy(attnTx[:], trAx[:])
                nc.vector.tensor_copy(attnTy[:], trAy[:])

            # ---- phase 2: out = proj(attn @ v) ----
            with (
                tc.tile_pool(name="p2", bufs=4) as p2p,
                tc.tile_pool(name="p2ps", bufs=8, space="PSUM") as p2ps,
            ):
                for t in range(RSTRIP // 2):
                    y0 = 2 * t
                    vb = y0 * PITCH + 1
                    # moving v: [part, 2 rows, 256] strided by PITCH
                    vAx = vX[0:64, vb:vb + 2 * PITCH].rearrange(
                        "p (r w) -> p r w", r=2)[:, :, 0:W]
                    vAy = vY[:, vb:vb + 2 * PITCH].rearrange(
                        "p (r w) -> p r w", r=2)[:, :, 0:W]
                    psA = p2ps.tile([128, 512], f32, tag="p2")
                    psB = p2ps.tile([128, 512], f32, tag="p2")
                    nc.tensor.matmul(psA[:, :], attnTx[0:64, 0:128], vAx,
                                     start=True, stop=False)
                    nc.tensor.matmul(psA[:, :], attnTy[:, 0:128], vAy,
                                     start=False, stop=True)
                    nc.tensor.matmul(psB[0:64, :], attnTx[0:64, 128:192],
                                     vAx, start=True, stop=False)
                    nc.tensor.matmul(psB[0:64, :], attnTy[:, 128:192], vAy,
                                     start=False, stop=True)
                    oh1 = p2p.tile([128, 512], bf, tag="oh")
                    oh2 = p2p.tile([128, 512], bf, tag="oh")
                    nc.vector.tensor_copy(oh1[:], psA[:])
                    nc.scalar.activation(oh2[0:64, :], psB[0:64, :], Copy)

                    psC = p2ps.tile([128, 512], f32, tag="p2")
                    psD = p2ps.tile([128, 512], f32, tag="p2")
                    nc.tensor.matmul(psC[:, :], wpA[:, 0:128], oh1[:],
                                     start=True, stop=False)
                    nc.tensor.matmul(psC[:, :], wpB[:, 0:128],
                                     oh2[0:64, :], start=False, stop=True)
                    nc.tensor.matmul(psD[0:64, :], wpA[:, 128:192],
                                     oh1[:], start=True, stop=False)
                    nc.tensor.matmul(psD[0:64, :], wpB[:, 128:192],
                                     oh2[0:64, :], start=False, stop=True)
                    o1 = p2p.tile([128, 512], f32, tag="o")
                    o2 = p2p.tile([128, 512], f32, tag="o")
                    nc.vector.tensor_copy(o1[:], psC[:])
                    nc.scalar.activation(o2[0:64, :], psD[0:64, :], Copy)
                    ob = y0 * W
                    nc.sync.dma_start(out_d[0:128, ob:ob + 512], o1[:])
                    nc.sync.dma_start(out_d[128:192, ob:ob + 512],
                                      o2[0:64, :])

    nc.compile()
    return nc


def _diag_blocks(w9, nch):
    """w9: [nch, 9] tap weights -> [nch, 9*nch] with diag blocks per tap."""
    out = np.zeros((nch, 9 * nch), np.float32)
    idx = np.arange(nch)
    for t in range(9):
        out[idx, t * nch + idx] = w9[:, t]
    return out.astype(BF)


def _prep_consts(wq, wq_dw, wkv, wkv_dw, wproj, temperature):
    w9q = wq_dw.reshape(C, 9).astype(np.float32)
    w9kv = wkv_dw.reshape(2 * C, 9).astype(np.float32)
    mask = np.full((C, C), MASK_NEG, np.float32)
    for h in range(HEADS):
        mask[h * CH:(h + 1) * CH, h * CH:(h + 1) * CH] = 0.0
    consts = {
        "wqT": np.ascontiguousarray(wq.T).astype(BF),
        "wkvT": np.ascontiguousarray(wkv.T).astype(BF),
        "wprojT": np.ascontiguousarray(wproj.T).astype(BF),
        "dq128": _diag_blocks(w9q[0:128], 128),
        "dq64": _diag_blocks(w9q[128:192], 64),
        "dk128": _diag_blocks(w9kv[0:128], 128),
        "dk64": _diag_blocks(w9kv[128:192], 64),
        "dv64": _diag_blocks(w9kv[192:256], 64),
        "dv128": _diag_blocks(w9kv[256:384], 128),
        "mask": mask,
        "temp_ch": np.repeat(
            np.asarray(temperature, np.float32).reshape(HEADS), CH
        ).reshape(C, 1).astype(np.float32),
    }
    return consts


def _get_runner():
    """Cached shard_map-jitted executable over the 8 cores.

    Mirrors concourse.bass2jax.run_bass_via_pjrt but builds the jit once so
    repeat calls don't re-trace.
    """
    if "runner" in _CACHE:
        return _CACHE["runner"]
    import jax
    import numpy as _np
    from jax.sharding import Mesh, PartitionSpec
    from jax.experimental.shard_map import shard_map
    from concourse import bass2jax, mybir

    nc = _CACHE["nc"]
    bass2jax.install_neuronx_cc_hook()

    partition_name = (
        nc.partition_id_tensor.name if nc.partition_id_tensor else None
    )
    in_names, out_names, out_avals, zero_outs = [], [], [], []
    for alloc in nc.m.functions[0].allocations:
        if not isinstance(alloc, mybir.MemoryLocationSet):
            continue
        name = alloc.memorylocations[0].name
        if alloc.kind == "ExternalInput":
            if name != partition_name:
                in_names.append(name)
        elif alloc.kind == "ExternalOutput":
            np_dt = mybir.dt.np(alloc.dtype)
            shape = tuple(alloc.tensor_shape)
            out_avals.append(jax.core.ShapedArray(shape, np_dt))
            out_names.append(name)
            zero_outs.append(_np.zeros(shape, np_dt))
    n_params = len(in_names)
    n_outs = len(out_names)
    all_in_names = list(in_names) + list(out_names)
    if partition_name is not None:
        all_in_names.append(partition_name)
    donate = tuple(range(n_params, n_params + n_outs))

    def _body(*args):
        operands = list(args)
        if partition_name is not None:
            operands.append(bass2jax.partition_id_tensor())
        outs = bass2jax._bass_exec_p.bind(
            *operands,
            out_avals=tuple(out_avals),
            in_names=tuple(all_in_names),
            out_names=tuple(out_names),
            lowering_input_output_aliases=(),
            sim_require_finite=True,
            sim_require_nnan=True,
            nc=nc,
        )
        return tuple(outs)

    devices = jax.devices()[:NCORES]
    mesh = Mesh(_np.asarray(devices), ("core",))
    in_specs = (PartitionSpec("core"),) * (n_params + n_outs)
    out_specs = (PartitionSpec("core"),) * len(out_names)
    sharded = jax.jit(
        shard_map(_body, mesh=mesh, in_specs=in_specs, out_specs=out_specs,
                  check_rep=False),
        donate_argnums=donate, keep_unused=True,
    )

    def run(in_maps):
        per_core = [[_np.asarray(m[nm]) for nm in in_names] for m in in_maps]
        concat_in = [
            _np.concatenate([per_core[c][i] for c in range(NCORES)], axis=0)
            for i in range(n_params)
        ]
        concat_zeros = [
            _np.zeros((NCORES * z.shape[0], *z.shape[1:]), z.dtype)
            for z in zero_outs
        ]
        out_arrs = sharded(*concat_in, *concat_zeros)
        return [
            {name: _np.asarray(out_arrs[i]).reshape(
                NCORES, *out_avals[i].shape)[c]
             for i, name in enumerate(out_names)}
            for c in range(NCORES)
        ]

    _CACHE["runner"] = run
    _CACHE["runner_parts"] = (sharded, in_names, out_names, out_avals,
                              zero_outs, mesh)
    return run


def kernel(x, x_ref, wq, wq_dw, wkv, wkv_dw, wproj, temperature):
    if "nc" not in _CACHE:
        _CACHE["nc"] = _build_program()

    x = np.asarray(x, np.float32)
    x_ref = np.asarray(x_ref, np.float32)
    consts = _prep_consts(
        np.asarray(wq, np.float32), np.asarray(wq_dw, np.float32),
        np.asarray(wkv, np.float32), np.asarray(wkv_dw, np.float32),
        np.asarray(wproj, np.float32), np.asarray(temperature, np.float32),
    )

    xp = np.pad(x, ((0, 0), (0, 0), (1, 1), (1, 1))).astype(BF)
    xrp = np.pad(x_ref, ((0, 0), (0, 0), (1, 1), (1, 1))).astype(BF)

    in_maps = []
    for core in range(NCORES):
        b, r = core // 4, core % 4
        m = dict(consts)
        m["x_strip"] = np.ascontiguousarray(
            xp[b, :, 64 * r:64 * r + 66, :]).reshape(C, 66 * PITCH)
        m["xr_strip"] = np.ascontiguousarray(
            xrp[b, :, 64 * r:64 * r + 66, :]).reshape(C, 66 * PITCH)
        in_maps.append(m)

    res = _get_runner()(in_maps)
    out = np.zeros((B, C, H, W), np.float32)
    for core in range(NCORES):
        b, r = core // 4, core % 4
        strip = np.asarray(res[core]["out_strip"], np.float32)
        out[b, :, 64 * r:64 * r + 64, :] = strip.reshape(C, 64, W)
    return out
